# revision 18
# baseline (speedup 1.0000x reference)
"""GIoU loss kernel for Trainium2, SPMD over 8 NeuronCores.

Problem: nn_LossGIoU (B=16, N=262144). loss = sum((1-giou)*m) / max(sum(m), 1)

Strategy
--------
- Shard the anchor axis N across the 8 cores (N/8 = 32768 anchors per core,
  all B=16 batches). Each core computes per-partition partial sums
  (sum(m*s), sum(m)) where s = iou + union/enclose, so that
  loss = (2*sum(m) - sum(m*s)) / max(sum(m), 1). Host combines partials.
- Host-side prep (part of sharding): de-interleave the [..., 4] box coords
  into planar components, cast to fp16 and scale all coordinate-valued
  tensors by 1/32 (GIoU is scale invariant; keeps fp16 in range). Planar
  fp16 stride-1 layout gives the DVE its 2x perf mode and halves DMA bytes.
- On-device per core: 4 groups of 4 batches. All elementwise tensor-tensor
  ops on DVE in fp16; exp/ln/relu/scaling on ACT (Exp+Ln share one table
  set; 1/x computed as exp(-ln(x)) since the ACT Reciprocal LUT is banned).
- Math per element (all coords pre-scaled, pcx/hw relative to anchors):
    hw = 0.5*exp(dw)*aw        (exp bias = -ln2)
    pc = dx*aw + acx
    wxu = min(hw, pc-t1) + min(hw, t2-pc)        # unclipped overlap width
    ew  = (tw - wxu) + 2*hw                      # enclose width (>=0 always)
    inter = relu(wxu)*relu(wyu); area1 = hwx*hwy; area2 = twx*twy
    union = 4*area1 + (area2 - inter)
    s = inter/union + union/enclose;  loss_elem = 2 - s
"""

import os
import numpy as np

B = 16
N = 262144
NCORES = 8
NC_N = N // NCORES      # 32768 anchors per core
P = 128                 # SBUF partitions
I = NC_N // P           # 256 boxes per partition per batch
G = 4                   # batch groups
BG = B // G             # batches per group
F = BG * I              # free elems per component per group (1024)
S = 1.0 / 32.0
LN2 = float(np.log(2.0))

_CACHE = {}

# Populated by the last kernel() call (for test harness introspection).
LAST_RESULTS = None


def _build_nc(reps=1):
    """Build the per-core BIR program. reps>1 wraps the whole compute in an
    on-device For_i loop (constant program size) for delta-method timing."""
    import concourse.bacc as bacc
    import concourse.mybir as mybir
    import concourse.tile as tile

    dt = mybir.dt
    Alu = mybir.AluOpType
    Act = mybir.ActivationFunctionType

    nc = bacc.Bacc(
        "TRN2",
        target_bir_lowering=False,
        debug=False,
    )

    # Register -ln2 as a const AP (same mechanism as Bass.__init__'s 0.0/1.0
    # consts: memset + barrier before any Tile scheduling), so activation's
    # bias lookup finds it without adding cross-engine tile deps.
    _c = nc.alloc_sbuf_tensor("const-float32-negln2", [128, 1], dt.float32)
    nc.gpsimd.memset(_c.ap(), -LN2)
    nc.const_aps.aps[(dt.float32, -LN2)] = _c.ap()
    nc.all_engine_barrier()

    data = nc.declare_dram_parameter(
        "data", [G, 9, BG, P, I], dt.float16, isOutput=False
    )
    anch = nc.declare_dram_parameter("anch", [4, P, I], dt.float16, isOutput=False)
    partials = nc.declare_dram_parameter(
        "partials", [P, 2 * G], dt.float32, isOutput=True
    )

    with tile.TileContext(nc) as tc:
        with (
            tc.tile_pool(name="const", bufs=1) as cpool,
            tc.tile_pool(name="inp", bufs=2) as ipool,
            tc.tile_pool(name="scr", bufs=1) as spool,
            tc.tile_pool(name="acc", bufs=1) as apool,
        ):
            v = nc.vector
            sc = nc.scalar

            # Anchor constants: load once, broadcast over the BG batches of a
            # group once (so every hot-loop operand is a flat stride-1 AP).
            anch_t = cpool.tile([P, 4 * I], dt.float16, bufs=1)
            nc.sync.dma_start(
                out=anch_t[:, :].rearrange("p (c i) -> p c i", c=4),
                in_=anch[:, :, :].transpose([1, 0, 2]),
            )
            aw4 = cpool.tile([P, F], dt.float16, bufs=1)
            ah4 = cpool.tile([P, F], dt.float16, bufs=1)
            acx4 = cpool.tile([P, F], dt.float16, bufs=1)
            acy4 = cpool.tile([P, F], dt.float16, bufs=1)
            for k, dst in enumerate([aw4, ah4, acx4, acy4]):
                src = anch_t[:, k * I : (k + 1) * I]
                sc.copy(
                    out=dst.rearrange("p (b i) -> p b i", b=BG),
                    in_=src.unsqueeze(1).broadcast_to((P, BG, I)),
                )

            sums = apool.tile([P, 2 * G], dt.float32, bufs=1)

            comp_names = ["dx", "dy", "dw", "dh", "tx1", "ty1", "tx2", "ty2", "mf"]

            def emit_group(g):
                comps = []
                for k, nm in enumerate(comp_names):
                    ct = ipool.tile(
                        [P, F], dt.float16, name=f"in_{nm}", tag=f"in_{nm}"
                    )
                    nc.sync.dma_start(
                        out=ct.rearrange("p (b i) -> p b i", b=BG),
                        in_=data[g, k, :, :, :].transpose([1, 0, 2]),
                    )
                    comps.append(ct)
                dx, dy, dw, dh, tx1, ty1, tx2, ty2, mf = comps

                def T(name, d=dt.float16):
                    return spool.tile([P, F], d, name=name, tag=name, bufs=1)

                # decode: half-widths and centers
                ex = T("ex")
                sc.activation(out=ex, in_=dw, func=Act.Exp, bias=-LN2)
                ey = T("ey")
                sc.activation(out=ey, in_=dh, func=Act.Exp, bias=-LN2)
                hwx = T("hwx")
                v.tensor_tensor(hwx, ex, aw4, Alu.mult)
                hwy = T("hwy")
                v.tensor_tensor(hwy, ey, ah4, Alu.mult)
                q1 = T("q1")
                v.tensor_tensor(q1, dx, aw4, Alu.mult)
                pcx = T("pcx")
                v.tensor_tensor(pcx, q1, acx4, Alu.add)
                q2 = T("q2")
                v.tensor_tensor(q2, dy, ah4, Alu.mult)
                pcy = T("pcy")
                v.tensor_tensor(pcy, q2, acy4, Alu.add)

                # overlap widths (unclipped): min(hw, pc-t1) + min(hw, t2-pc)
                ux = T("ux")
                v.tensor_tensor(ux, pcx, tx1, Alu.subtract)
                vx = T("vx")
                v.tensor_tensor(vx, tx2, pcx, Alu.subtract)
                m1 = T("m1")
                v.tensor_tensor(m1, hwx, ux, Alu.min)
                m2 = T("m2")
                v.tensor_tensor(m2, hwx, vx, Alu.min)
                wxu = T("wxu")
                v.tensor_tensor(wxu, m1, m2, Alu.add)
                uy = T("uy")
                v.tensor_tensor(uy, pcy, ty1, Alu.subtract)
                vy = T("vy")
                v.tensor_tensor(vy, ty2, pcy, Alu.subtract)
                m3 = T("m3")
                v.tensor_tensor(m3, hwy, uy, Alu.min)
                m4 = T("m4")
                v.tensor_tensor(m4, hwy, vy, Alu.min)
                wyu = T("wyu")
                v.tensor_tensor(wyu, m3, m4, Alu.add)

                wxr = T("wxr")
                sc.activation(out=wxr, in_=wxu, func=Act.Relu)
                wyr = T("wyr")
                sc.activation(out=wyr, in_=wyu, func=Act.Relu)

                # widths / areas
                twx = T("twx")
                v.tensor_tensor(twx, tx2, tx1, Alu.subtract)
                twy = T("twy")
                v.tensor_tensor(twy, ty2, ty1, Alu.subtract)
                area2 = T("area2")
                v.tensor_tensor(area2, twx, twy, Alu.mult)
                inter = T("inter")
                v.tensor_tensor(inter, wxr, wyr, Alu.mult)
                area1 = T("area1")
                v.tensor_tensor(area1, hwx, hwy, Alu.mult)
                area1q = T("area1q")
                sc.mul(out=area1q, in_=area1, mul=4.0)

                # enclose widths: (tw - wxu) + 2*hw  (always >= 0)
                pwx = T("pwx")
                v.tensor_scalar(pwx, hwx, 2.0, None, Alu.mult)
                pwy = T("pwy")
                v.tensor_scalar(pwy, hwy, 2.0, None, Alu.mult)
                t1x = T("t1x")
                v.tensor_tensor(t1x, twx, wxu, Alu.subtract)
                ewx = T("ewx")
                v.tensor_tensor(ewx, t1x, pwx, Alu.add)
                t1y = T("t1y")
                v.tensor_tensor(t1y, twy, wyu, Alu.subtract)
                ewy = T("ewy")
                v.tensor_tensor(ewy, t1y, pwy, Alu.add)
                enclose = T("enclose")
                v.tensor_tensor(enclose, ewx, ewy, Alu.mult)

                # union = 4*area1 + (area2 - inter)
                u1 = T("u1")
                v.tensor_tensor(u1, area2, inter, Alu.subtract)
                union = T("union")
                v.tensor_tensor(union, area1q, u1, Alu.add)

                # reciprocals via exp(-ln(x)) on ACT (fp32 intermediates)
                lnu = T("lnu", dt.float32)
                sc.activation(out=lnu, in_=union, func=Act.Ln)
                lne = T("lne", dt.float32)
                sc.activation(out=lne, in_=enclose, func=Act.Ln)
                r1 = T("r1")
                sc.activation(out=r1, in_=lnu, func=Act.Exp, scale=-1.0)
                r2 = T("r2")
                sc.activation(out=r2, in_=lne, func=Act.Exp, scale=-1.0)

                a = T("a")
                v.tensor_tensor(a, inter, r1, Alu.mult)
                bq = T("bq")
                v.tensor_tensor(bq, union, r2, Alu.mult)
                s = T("s")
                v.tensor_tensor(s, a, bq, Alu.add)

                # masked sum: sums[:, g] = sum_f m*s ; count: sums[:, G+g] = sum_f m
                ms = T("ms")
                v.tensor_tensor(ms, s, mf, Alu.mult)
                jv = T("jv")
                v.tensor_scalar(
                    jv[:, :], ms, 1.0, None, Alu.mult, Alu.add,
                    accum_out=sums[:, g : g + 1],
                )
                ja = T("ja")
                sc.activation(
                    out=ja,
                    in_=mf,
                    func=Act.Copy,
                    accum_out=sums[:, G + g : G + g + 1],
                )

            if reps == 1:
                for g in range(G):
                    emit_group(g)
            else:
                with tc.For_i(0, reps, 1):
                    for g in range(G):
                        emit_group(g)

            nc.sync.dma_start(out=partials[:, :], in_=sums[:, :])

    nc.compile()
    return nc


def _prep_core_inputs(pred_deltas, target_boxes, anchors, pos_mask):
    """Host-side shard prep: slice per core, de-interleave coords to planar
    fp16, scale coordinate-valued tensors by S. Returns list of in_maps."""
    # planar fp16 full-tensor views/copies
    pred16 = pred_deltas.astype(np.float16)  # deltas are unitless: no scaling
    tgt16 = (target_boxes * np.float32(S)).astype(np.float16)
    mask16 = pos_mask.astype(np.float16)

    anc = anchors.astype(np.float32)
    aw = (anc[:, 2] - anc[:, 0]) * np.float32(S)
    ah = (anc[:, 3] - anc[:, 1]) * np.float32(S)
    acx = (anc[:, 0] + anc[:, 2]) * np.float32(0.5 * S)
    acy = (anc[:, 1] + anc[:, 3]) * np.float32(0.5 * S)
    anch_all = np.stack([aw, ah, acx, acy], axis=0).astype(np.float16)  # [4, N]

    in_maps = []
    for c in range(NCORES):
        sl = slice(c * NC_N, (c + 1) * NC_N)
        # [B, NC_N] -> [G, BG, P, I]
        def shp(x):
            return np.ascontiguousarray(x).reshape(G, BG, P, I)

        comps = [shp(pred16[:, sl, k]) for k in range(4)]
        comps += [shp(tgt16[:, sl, k]) for k in range(4)]
        comps += [shp(mask16[:, sl])]
        data_c = np.ascontiguousarray(np.stack(comps, axis=1))  # [G,9,BG,P,I]
        anch_c = np.ascontiguousarray(anch_all[:, sl].reshape(4, P, I))
        in_maps.append({"data": data_c, "anch": anch_c})
    return in_maps


def kernel(pred_deltas, target_boxes, anchors, pos_mask):
    global LAST_RESULTS
    from concourse.bass_utils import run_bass_kernel_spmd

    assert pred_deltas.shape == (B, N, 4), pred_deltas.shape
    assert pos_mask.shape == (B, N), pos_mask.shape

    if "nc" not in _CACHE:
        _CACHE["nc"] = _build_nc(reps=1)
    nc = _CACHE["nc"]

    in_maps = _prep_core_inputs(pred_deltas, target_boxes, anchors, pos_mask)
    trace = bool(int(os.environ.get("GIOU_TRACE", "0")))
    res = run_bass_kernel_spmd(nc, in_maps, list(range(NCORES)), trace=trace)
    LAST_RESULTS = res

    total_ms = 0.0
    total_m = 0.0
    for r in res.results:
        p = r["partials"].astype(np.float64)
        total_ms += p[:, :G].sum()
        total_m += p[:, G:].sum()
    loss = (2.0 * total_m - total_ms) / max(total_m, 1.0)
    return np.float32(loss)
